# revision 35
# baseline (speedup 1.0000x reference)
"""Trainium2 Bass kernel for nn_AttentionMLP (B=4, S=4096, two attention+MLP stages).

Sharding: 8 cores = 4 batches x 2 sequence-halves. Each core computes its
2048 query rows end-to-end; pairwise AllGathers (chunked, pipelined)
exchange the stage-1 output halves so stage 2 attends over the full
sequence.

Layout strategy (per core, all feature-major / transposed):
  xT [64, S]   -> qT/kT [64, *] projections on PE (fp32r)
  scoresT[j, si] blocks via PE (K=64), exp on ACT into SBUF (fp32r)
  attn@v + rowsum fused: lhsT = [v | ones] [128jb, 65], accumulate in PSUM
  normalize via reciprocal_approx_fast + gpsimd partition_broadcast + DVE mul
  MLP: W1T/W2T matmuls, ELU = max(x+b,0) + exp(min(x+b,0)) - 1 (the -1 is
  folded into the next layer's bias), biases via K=1 ones-matmul into PSUM.

All weights ship in one packed DRAM tensor (single DMA): DMA dispatch costs
~650ns of sequencer time each, so count matters more than bytes here.
"""

import numpy as np
from contextlib import ExitStack

import concourse.bass as bass
import concourse.tile as tile
from concourse import bacc, mybir
from concourse import bass_utils

F32 = mybir.dt.float32
F32R = mybir.dt.float32r
EXP = mybir.ActivationFunctionType.Exp
ADD = mybir.AluOpType.add
MIN = mybir.AluOpType.min
MAX = mybir.AluOpType.max

N_CORES = 8
B, S, D = 4, 4096, 64
R = S // 2            # own query rows per core
HD = 256
NCK = R // 512        # si-chunks per core (4 x 512)
NJB = S // 128        # key blocks (32 x 128)
# exp-group sizes per chunk: one double-buffered [128, 1536] scores tag
# (6 banks) + av (1) + mlp (1).
GROUPS = [3] * 10 + [2]
assert sum(GROUPS) == NJB

# packed-weight column layout (f32 words per partition)
# region A (partitions 0-63, one 448-col block per stage): wq|wk|wv|w1t
WQ0, WK0, WV0, W1T0 = 0, 64, 128, 192
RA = 896
# region B (all 128 partitions): w2t (2 stages x 2 K-blocks x 64) |
# b1c (2 stages x 2 cols) | b2 rows (2 stages x 64, partition 0 only)
W2T0, B1C0, B2R0 = RA, RA + 256, RA + 260
WCOLS = RA + 260 + 128


def make_proj(nc, pools, sfx, xT, q_src, ptag="sA", pbufs=2):
    """Allocate a stage's projection tiles and return them with a per-slice
    emitter. Double-buffered tags so the next stage's projections can be
    emitted while the previous stage still reads its own."""
    sb, ps, wt, ones512, ones128 = pools
    wsl = wt[0:64, sfx * 448:sfx * 448 + 448]
    qT = sb.tile([64, R], F32R, tag="qT", bufs=2, name=f"qT{sfx}")
    kT = sb.tile([64, S], F32R, tag="kT", bufs=2, name=f"kT{sfx}")
    v_aug = sb.tile([128, NJB, 65], F32R, tag="v_aug", bufs=2,
                    name=f"v_aug{sfx}")
    onescol = sb.tile([128, NJB], F32, tag="onescol", bufs=2,
                      name=f"onescol{sfx}")
    nc.vector.memset(onescol[:], 1.0)
    nc.vector.tensor_copy(v_aug[:, :, 64:65], onescol[:].unsqueeze(2))

    def emit_proj(n):
        sl = slice(n * 512, (n + 1) * 512)
        pk = ps.tile([64, 512], F32, tag=ptag, bufs=pbufs)
        nc.tensor.matmul(pk[:], wsl[:, WK0:WK0 + 64], xT[:, sl],
                         start=True, stop=True)
        nc.vector.tensor_copy(kT[:, sl], pk[:])
        if n < R // 512:
            pq = ps.tile([64, 512], F32, tag=ptag, bufs=pbufs)
            nc.tensor.matmul(pq[:], wsl[:, WQ0:WQ0 + 64], q_src[:, sl],
                             start=True, stop=True)
            nc.vector.tensor_copy(qT[:, sl], pq[:])
        pv = ps.tile([128, 4, 64], F32, tag=ptag, bufs=pbufs)
        for i in range(4):
            jb = n * 4 + i
            nc.tensor.matmul(pv[:, i, :], xT[:, jb * 128:(jb + 1) * 128],
                             wsl[:, WV0:WV0 + 64], start=True, stop=True)
        nc.vector.tensor_copy(v_aug[:, n * 4:(n + 1) * 4, 0:64], pv[:])

    return qT, kT, v_aug, emit_proj


def _stage(nc, pools, sfx, proj, own_emits, outT=None, out_dram=None,
           out_chunk_hook=None, tail_emits=()):
    """One attention+MLP stage. proj = make_proj(...) result. own_emits:
    projection slices to emit in this stage's first chunk; tail_emits:
    callables (e.g. the next stage's projections) interleaved into the
    last chunk. Writes outT (SBUF, stage 1) or out_dram (stage 2)."""
    sb, ps, wt, ones512, ones128 = pools
    wsl = wt[0:64, sfx * 448:sfx * 448 + 448]
    w2t = wt[:, W2T0 + sfx * 128:W2T0 + sfx * 128 + 128]
    b1c = wt[:, B1C0 + sfx * 2:B1C0 + sfx * 2 + 2].bitcast(F32)
    b2 = wt[0:1, B2R0 + sfx * 64:B2R0 + sfx * 64 + 64]
    qT, kT, v_aug, emit_proj = proj

    # --- per si-chunk: scores -> exp -> attn@v -> normalize -> MLP ---
    for n in range(NCK):
        qs = qT[:, n * 512:(n + 1) * 512]
        av = None
        jb = 0
        for gi, gsz in enumerate(GROUPS):
            if n == 0 and gi < len(own_emits):
                emit_proj(own_emits[gi])
            if n == NCK - 1 and gi < len(tail_emits):
                tail_emits[gi]()
            st = ps.tile([128, gsz * 512], F32, tag="sA", bufs=2)
            for i in range(gsz):
                nc.tensor.matmul(st[:, i * 512:(i + 1) * 512],
                                 kT[:, (jb + i) * 128:(jb + i + 1) * 128],
                                 qs, start=True, stop=True)
            ex = sb.tile([128, gsz * 512], F32R, tag="exp", bufs=4)
            nc.scalar.activation(ex[:], st[:], EXP)
            if av is None:
                av = ps.tile([65, 512], F32, tag="av", bufs=1)
            for i in range(gsz):
                nc.tensor.matmul(av[:], v_aug[:, jb + i, :],
                                 ex[:, i * 512:(i + 1) * 512],
                                 start=(jb + i == 0), stop=(jb + i == NJB - 1))
            jb += gsz

        # normalize: aT = av[0:64] / av[64]. Drain av with two immediate
        # copies so its PSUM bank frees for the next chunk's attn@v; the
        # recip/broadcast/mul then run off av's critical path.
        # (partition_broadcast and the custom recip op only work from
        # partition 0; DVE copies handle the cross-partition move.)
        rs = sb.tile([1, 512], F32, tag="rs", bufs=3)
        nc.vector.tensor_copy(rs[:], av[64:65, :])
        araw = sb.tile([64, 512], F32, tag="araw", bufs=3)
        nc.vector.tensor_copy(araw[:], av[0:64, :])
        rr = sb.tile([1, 512], F32, tag="rr", bufs=3)
        nc.vector.reciprocal_approx_fast(rr[:], rs[:])
        rb = sb.tile([64, 512], F32, tag="rb", bufs=3)
        nc.gpsimd.partition_broadcast(rb[:], rr[:])
        aT = sb.tile([64, 512], F32R, tag="aT", bufs=3)
        nc.vector.tensor_mul(aT[:], araw[:], rb[:])

        # MLP hidden: hT = elu(W1 @ aT + b1) + 1  (the -1 lives in b2_eff)
        u = sb.tile([128, 1024], F32, tag="u", bufs=2)
        r = sb.tile([128, 1024], F32, tag="r", bufs=2)
        for j in range(2):
            ph = ps.tile([128, 512], F32, tag="mlp", bufs=1)
            nc.tensor.matmul(ph[:], wsl[:, W1T0 + j * 128:W1T0 + (j + 1) * 128],
                             aT[:], start=True, stop=True)
            nc.vector.tensor_scalar(u[:, j * 512:(j + 1) * 512], ph[:],
                                    b1c[:, j:j + 1], 0.0, op0=ADD, op1=MIN)
            nc.vector.tensor_scalar(r[:, j * 512:(j + 1) * 512], ph[:],
                                    b1c[:, j:j + 1], 0.0, op0=ADD, op1=MAX)
        e = sb.tile([128, 1024], F32, tag="e", bufs=2)
        nc.scalar.activation(e[:], u[:], EXP)
        hT = sb.tile([128, 1024], F32R, tag="hT", bufs=3)
        nc.vector.tensor_add(hT[:], r[:], e[:])

        # output projection (+ bias via K=1 ones-matmul)
        if outT is not None:
            po = ps.tile([64, 512], F32, tag="mlp", bufs=1)
            nc.tensor.matmul(po[:], b2[:], ones512[:], start=True, stop=False)
            for j in range(2):
                nc.tensor.matmul(po[:], w2t[:, j * 64:(j + 1) * 64],
                                 hT[:, j * 512:(j + 1) * 512],
                                 start=False, stop=(j == 1))
            nc.vector.tensor_copy(outT[:, n * 512:(n + 1) * 512], po[:])
            if out_chunk_hook is not None:
                out_chunk_hook(n)
        else:
            # stage 2: emit row-major [si, 64] directly to DRAM
            for ss in range(4):
                po2 = ps.tile([128, 64], F32, tag="mlp", bufs=1)
                nc.tensor.matmul(po2[:], ones128[:], b2[:],
                                 start=True, stop=False)
                for j in range(2):
                    nc.tensor.matmul(
                        po2[:],
                        hT[:, j * 512 + ss * 128:j * 512 + (ss + 1) * 128],
                        w2t[:, j * 64:(j + 1) * 64],
                        start=False, stop=(j == 1))
                fin = sb.tile([128, 64], F32, tag="fin", bufs=3)
                nc.vector.tensor_copy(fin[:], po2[:])
                row0 = n * 512 + ss * 128
                nc.sync.dma_start(out_dram[row0:row0 + 128, :], fin[:])


def build_nc(n_cores=N_CORES, reps=1, exch_chunks=NCK):
    nc = bacc.Bacc("TRN2", target_bir_lowering=False, debug=False,
                   num_devices=n_cores)

    xT_d = nc.dram_tensor("xT", [64, S], F32R, kind="ExternalInput").ap()
    w_d = nc.dram_tensor("wpack", [128, WCOLS], F32R,
                         kind="ExternalInput").ap()
    out_d = nc.dram_tensor("out1", [R, 64], F32, kind="ExternalOutput").ap()

    with tile.TileContext(nc) as tc, ExitStack() as ctx:
        consts = ctx.enter_context(tc.tile_pool(name="consts", bufs=1))
        sb = ctx.enter_context(tc.tile_pool(name="sb", bufs=1))
        ps = ctx.enter_context(tc.tile_pool(name="ps", bufs=2, space="PSUM"))
        dram = ctx.enter_context(tc.tile_pool(name="dram", bufs=1,
                                              space="DRAM"))

        wt = consts.tile([128, WCOLS], F32R)
        nc.sync.dma_start(wt[:, 0:448], w_d[:, 0:448])
        nc.scalar.dma_start(wt[:, 448:WCOLS], w_d[:, 448:WCOLS])
        ones_f32 = consts.tile([1, 512], F32)
        nc.vector.memset(ones_f32[:], 1.0)
        ones512 = consts.tile([1, 512], F32R)
        nc.vector.tensor_copy(ones512[:], ones_f32[:])
        ones128 = consts.tile([1, 128], F32R)
        nc.vector.tensor_copy(ones128[:], ones_f32[:, 0:128])
        pools = (sb, ps, wt[:], ones512, ones128)

        # spread big loads across the three DMA-dispatch queues
        dma_engines = [nc.sync, nc.scalar, nc.gpsimd]

        for _rep in range(reps):
            xT = sb.tile([64, S], F32R, tag="xt", bufs=2, name=f"xT_{_rep}")
            for n in range(S // 512):
                dma_engines[n % 3].dma_start(
                    xT[:, n * 512:(n + 1) * 512],
                    xT_d[:, n * 512:(n + 1) * 512])

            outT = sb.tile([64, R], F32R, tag="outT")
            xT2 = sb.tile([64, S], F32R, tag="xt", bufs=2, name=f"xT2_{_rep}")
            csz = R // exch_chunks
            bounce_ins = [dram.tile([64, csz], F32R,
                                    name=f"bi_{_rep}_{n}", tag=f"bi{n}")
                          for n in range(exch_chunks)]
            bounce_outs = [dram.tile([2, 64, csz], F32R,
                                     name=f"bo_{_rep}_{n}", tag=f"bo{n}")
                           for n in range(exch_chunks)]

            def exchange_chunk(n):
                # fire the exchange for every bounce chunk fully covered by
                # the just-finished outT chunk n (chunk size 512), then pull
                # both gathered halves straight into xT2
                for e in range(exch_chunks):
                    if (e + 1) * csz <= (n + 1) * 512 and \
                            (e + 1) * csz > n * 512:
                        sl = slice(e * csz, (e + 1) * csz)
                        nc.sync.dma_start(bounce_ins[e][:], outT[:, sl])
                        if n_cores > 1:
                            nc.gpsimd.collective_compute(
                                "AllGather", mybir.AluOpType.bypass,
                                replica_groups=[[0, 1], [2, 3],
                                                [4, 5], [6, 7]],
                                ins=[bounce_ins[e][:].opt()],
                                outs=[bounce_outs[e][:].opt()])
                        else:
                            for m in range(2):
                                nc.sync.dma_start(bounce_outs[e][m],
                                                  bounce_ins[e][:])
                        for m in range(2):
                            dma_engines[(m * exch_chunks + e) % 3].dma_start(
                                xT2[:, m * R + e * csz:m * R + (e + 1) * csz],
                                bounce_outs[e][m])

            proj1 = make_proj(nc, pools, 0, xT[:], xT[:, 0:R])
            proj2 = make_proj(nc, pools, 1, xT2[:], outT[:], ptag="mlp", pbufs=1)
            emit2 = proj2[3]
            # stage-2 projection slices that only need exchange chunks 0-2
            # are emitted inside stage-1's last chunk; slices 3 and 7 need
            # the final exchange and run at stage-2's start.
            tail = [lambda s=s: emit2(s) for s in (0, 1, 2, 4, 5, 6)]
            _stage(nc, pools, 0, proj1, list(range(8)), outT=outT,
                   out_chunk_hook=exchange_chunk, tail_emits=tail)
            _stage(nc, pools, 1, proj2, [3, 7], out_dram=out_d)

    nc.compile()
    return nc


def prep_inputs(x, q, k, v, q1, k1, v1, W1, b1, W2, b2, W11, b11, W22, b22):
    """Returns per-core in_maps for run_bass_kernel_spmd."""
    f = np.float32

    def cast(a):
        return np.ascontiguousarray(np.asarray(a), dtype=f)

    scale = f(1.0 / np.sqrt(np.float32(64)))
    wpack = np.zeros((128, WCOLS), dtype=f)
    for sfx, (qq, kk, vv, W1_, b1_, W2_, b2_) in enumerate(
            [(q, k, v, W1, b1, W2, b2), (q1, k1, v1, W11, b11, W22, b22)]):
        c0 = 448 * sfx
        wpack[0:64, c0 + WQ0:c0 + WQ0 + 64] = cast(qq) * scale
        wpack[0:64, c0 + WK0:c0 + WK0 + 64] = cast(kk)
        wpack[0:64, c0 + WV0:c0 + WV0 + 64] = cast(vv)
        wpack[0:64, c0 + W1T0:c0 + W1T0 + HD] = cast(W1_).T
        w2T = cast(W2_).T                                 # [HD, 64]
        for j in range(2):
            wpack[:, W2T0 + sfx * 128 + j * 64:
                  W2T0 + sfx * 128 + (j + 1) * 64] = w2T[j * 128:(j + 1) * 128]
            wpack[:, B1C0 + sfx * 2 + j] = cast(b1_)[j * 128:(j + 1) * 128]
        wpack[0, B2R0 + sfx * 64:B2R0 + (sfx + 1) * 64] = \
            cast(b2_) - cast(W2_).sum(axis=1)

    in_maps = []
    xc = cast(x)
    for c in range(N_CORES):
        b, h = c // 2, c % 2
        xb = xc[b]                      # [S, 64]
        if h == 1:                      # own half first
            xb = np.concatenate([xb[R:], xb[:R]], axis=0)
        in_maps.append({"xT": np.ascontiguousarray(xb.T), "wpack": wpack})
    return in_maps


_NC_CACHE = None


def kernel(**inputs) -> np.ndarray:
    global _NC_CACHE
    if _NC_CACHE is None:
        _NC_CACHE = build_nc()
    nc = _NC_CACHE
    in_maps = prep_inputs(**inputs)
    res = bass_utils.run_bass_kernel_spmd(nc, in_maps,
                                          core_ids=list(range(N_CORES)))
    out = np.empty((B, S, 64), dtype=np.float32)
    for c in range(N_CORES):
        b, h = c // 2, c % 2
        out[b, h * R:(h + 1) * R, :] = res.results[c]["out1"]
    return out


# revision 36
# speedup vs baseline: 1.2991x; 1.2991x over previous
"""Trainium2 Bass kernel for nn_AttentionMLP (B=4, S=4096, two attention+MLP stages).

Sharding: 8 cores = 4 batches x 2 sequence-halves. Each core computes its
2048 query rows end-to-end; pairwise AllGathers (chunked, pipelined)
exchange the stage-1 output halves so stage 2 attends over the full
sequence.

Layout strategy (per core, all feature-major / transposed):
  xT [64, S]   -> qT/kT [64, *] projections on PE (fp32r)
  scoresT[j, si] blocks via PE (K=64), exp on ACT into SBUF (fp32r)
  attn@v + rowsum fused: lhsT = [v | ones] [128jb, 65], accumulate in PSUM
  normalize via reciprocal_approx_fast + gpsimd partition_broadcast + DVE mul
  MLP: W1T/W2T matmuls, ELU = max(x+b,0) + exp(min(x+b,0)) - 1 (the -1 is
  folded into the next layer's bias), biases via K=1 ones-matmul into PSUM.

All weights ship in one packed DRAM tensor (single DMA): DMA dispatch costs
~650ns of sequencer time each, so count matters more than bytes here.
"""

import numpy as np
from contextlib import ExitStack

import concourse.bass as bass
import concourse.tile as tile
from concourse import bacc, mybir
from concourse import bass_utils

F32 = mybir.dt.float32
F32R = mybir.dt.float32r
EXP = mybir.ActivationFunctionType.Exp
ADD = mybir.AluOpType.add
MIN = mybir.AluOpType.min
MAX = mybir.AluOpType.max

N_CORES = 8
B, S, D = 4, 4096, 64
R = S // 2            # own query rows per core
HD = 256
NCK = R // 512        # si-chunks per core (4 x 512)
NJB = S // 128        # key blocks (32 x 128)
# exp-group sizes per chunk: one double-buffered [128, 1536] scores tag
# (6 banks) + av (1) + mlp (1).
GROUPS = [3] * 10 + [2]
assert sum(GROUPS) == NJB

# packed-weight column layout (f32 words per partition)
# region A (partitions 0-63, one 448-col block per stage): wq|wk|wv|w1t
WQ0, WK0, WV0, W1T0 = 0, 64, 128, 192
RA = 896
# region B (all 128 partitions): w2t (2 stages x 2 K-blocks x 64) |
# b1c (2 stages x 2 cols) | b2 rows (2 stages x 64, partition 0 only)
W2T0, B1C0, B2R0 = RA, RA + 256, RA + 260
WCOLS = RA + 260 + 128


def make_proj(nc, pools, sfx, xT, q_src, ptag="sA", pbufs=2):
    """Allocate a stage's projection tiles and return them with a per-slice
    emitter. Double-buffered tags so the next stage's projections can be
    emitted while the previous stage still reads its own."""
    sb, ps, wt, ones512, ones128 = pools
    wsl = wt[0:64, sfx * 448:sfx * 448 + 448]
    qT = sb.tile([64, R], F32R, tag="qT", bufs=2, name=f"qT{sfx}")
    kT = sb.tile([64, S], F32R, tag="kT", bufs=2, name=f"kT{sfx}")
    v_aug = sb.tile([128, NJB, 65], F32R, tag="v_aug", bufs=2,
                    name=f"v_aug{sfx}")
    onescol = sb.tile([128, NJB], F32, tag="onescol", bufs=2,
                      name=f"onescol{sfx}")
    nc.vector.memset(onescol[:], 1.0)
    nc.vector.tensor_copy(v_aug[:, :, 64:65], onescol[:].unsqueeze(2))

    def emit_proj(n):
        sl = slice(n * 512, (n + 1) * 512)
        pk = ps.tile([64, 512], F32, tag=ptag, bufs=pbufs)
        nc.tensor.matmul(pk[:], wsl[:, WK0:WK0 + 64], xT[:, sl],
                         start=True, stop=True)
        nc.vector.tensor_copy(kT[:, sl], pk[:])
        if n < R // 512:
            pq = ps.tile([64, 512], F32, tag=ptag, bufs=pbufs)
            nc.tensor.matmul(pq[:], wsl[:, WQ0:WQ0 + 64], q_src[:, sl],
                             start=True, stop=True)
            nc.vector.tensor_copy(qT[:, sl], pq[:])
        pv = ps.tile([128, 4, 64], F32, tag=ptag, bufs=pbufs)
        for i in range(4):
            jb = n * 4 + i
            nc.tensor.matmul(pv[:, i, :], xT[:, jb * 128:(jb + 1) * 128],
                             wsl[:, WV0:WV0 + 64], start=True, stop=True)
        nc.vector.tensor_copy(v_aug[:, n * 4:(n + 1) * 4, 0:64], pv[:])

    return qT, kT, v_aug, emit_proj


def _stage(nc, pools, sfx, proj, own_emits, outT=None, out_dram=None,
           out_chunk_hook=None, tail_emits=()):
    """One attention+MLP stage. proj = make_proj(...) result. own_emits:
    projection slices to emit in this stage's first chunk; tail_emits:
    callables (e.g. the next stage's projections) interleaved into the
    last chunk. Writes outT (SBUF, stage 1) or out_dram (stage 2)."""
    sb, ps, wt, ones512, ones128 = pools
    wsl = wt[0:64, sfx * 448:sfx * 448 + 448]
    w2t = wt[:, W2T0 + sfx * 128:W2T0 + sfx * 128 + 128]
    b1c = wt[:, B1C0 + sfx * 2:B1C0 + sfx * 2 + 2].bitcast(F32)
    b2 = wt[0:1, B2R0 + sfx * 64:B2R0 + sfx * 64 + 64]
    qT, kT, v_aug, emit_proj = proj

    # --- per si-chunk: scores -> exp -> attn@v -> normalize -> MLP ---
    for n in range(NCK):
        qs = qT[:, n * 512:(n + 1) * 512]
        av = None
        jb = 0
        for gi, gsz in enumerate(GROUPS):
            if n == 0 and gi < len(own_emits):
                emit_proj(own_emits[gi])
            if n == NCK - 1 and gi < len(tail_emits):
                tail_emits[gi]()
            st = ps.tile([128, gsz * 512], F32, tag="sA", bufs=2)
            for i in range(gsz):
                nc.tensor.matmul(st[:, i * 512:(i + 1) * 512],
                                 kT[:, (jb + i) * 128:(jb + i + 1) * 128],
                                 qs, start=True, stop=True)
            ex = sb.tile([128, gsz * 512], F32R, tag="exp", bufs=4)
            nc.scalar.activation(ex[:], st[:], EXP)
            if av is None:
                av = ps.tile([65, 512], F32, tag="av", bufs=1)
            for i in range(gsz):
                nc.tensor.matmul(av[:], v_aug[:, jb + i, :],
                                 ex[:, i * 512:(i + 1) * 512],
                                 start=(jb + i == 0), stop=(jb + i == NJB - 1))
            jb += gsz

        # normalize: aT = av[0:64] / av[64]. Drain av with two immediate
        # copies so its PSUM bank frees for the next chunk's attn@v; the
        # recip/broadcast/mul then run off av's critical path.
        # (partition_broadcast and the custom recip op only work from
        # partition 0; DVE copies handle the cross-partition move.)
        rs = sb.tile([1, 512], F32, tag="rs", bufs=3)
        nc.vector.tensor_copy(rs[:], av[64:65, :])
        araw = sb.tile([64, 512], F32, tag="araw", bufs=3)
        nc.vector.tensor_copy(araw[:], av[0:64, :])
        rr = sb.tile([1, 512], F32, tag="rr", bufs=3)
        nc.vector.reciprocal_approx_fast(rr[:], rs[:])
        rb = sb.tile([64, 512], F32, tag="rb", bufs=3)
        nc.gpsimd.partition_broadcast(rb[:], rr[:])
        aT = sb.tile([64, 512], F32R, tag="aT", bufs=3)
        nc.vector.tensor_mul(aT[:], araw[:], rb[:])

        # MLP hidden: hT = elu(W1 @ aT + b1) + 1  (the -1 lives in b2_eff)
        u = sb.tile([128, 1024], F32, tag="u", bufs=2)
        r = sb.tile([128, 1024], F32, tag="r", bufs=2)
        for j in range(2):
            ph = ps.tile([128, 512], F32, tag="mlp", bufs=1)
            nc.tensor.matmul(ph[:], wsl[:, W1T0 + j * 128:W1T0 + (j + 1) * 128],
                             aT[:], start=True, stop=True)
            nc.vector.tensor_scalar(u[:, j * 512:(j + 1) * 512], ph[:],
                                    b1c[:, j:j + 1], 0.0, op0=ADD, op1=MIN)
            nc.vector.tensor_scalar(r[:, j * 512:(j + 1) * 512], ph[:],
                                    b1c[:, j:j + 1], 0.0, op0=ADD, op1=MAX)
        e = sb.tile([128, 1024], F32, tag="e", bufs=2)
        nc.scalar.activation(e[:], u[:], EXP)
        hT = sb.tile([128, 1024], F32R, tag="hT", bufs=3)
        nc.vector.tensor_add(hT[:], r[:], e[:])

        # output projection (+ bias via K=1 ones-matmul)
        if outT is not None:
            po = ps.tile([64, 512], F32, tag="mlp", bufs=1)
            nc.tensor.matmul(po[:], b2[:], ones512[:], start=True, stop=False)
            for j in range(2):
                nc.tensor.matmul(po[:], w2t[:, j * 64:(j + 1) * 64],
                                 hT[:, j * 512:(j + 1) * 512],
                                 start=False, stop=(j == 1))
            nc.vector.tensor_copy(outT[:, n * 512:(n + 1) * 512], po[:])
            if out_chunk_hook is not None:
                out_chunk_hook(n)
        else:
            # stage 2: emit row-major [si, 64] directly to DRAM
            for ss in range(4):
                po2 = ps.tile([128, 64], F32, tag="mlp", bufs=1)
                nc.tensor.matmul(po2[:], ones128[:], b2[:],
                                 start=True, stop=False)
                for j in range(2):
                    nc.tensor.matmul(
                        po2[:],
                        hT[:, j * 512 + ss * 128:j * 512 + (ss + 1) * 128],
                        w2t[:, j * 64:(j + 1) * 64],
                        start=False, stop=(j == 1))
                fin = sb.tile([128, 64], F32, tag="fin", bufs=3)
                nc.vector.tensor_copy(fin[:], po2[:])
                row0 = n * 512 + ss * 128
                nc.sync.dma_start(out_dram[row0:row0 + 128, :], fin[:])


def build_nc(n_cores=N_CORES, reps=1, exch_chunks=NCK):
    nc = bacc.Bacc("TRN2", target_bir_lowering=False, debug=False,
                   num_devices=n_cores)

    xT_d = nc.dram_tensor("xT", [64, S], F32R, kind="ExternalInput").ap()
    w_d = nc.dram_tensor("wpack", [128, WCOLS], F32R,
                         kind="ExternalInput").ap()
    out_d = nc.dram_tensor("out1", [R, 64], F32, kind="ExternalOutput").ap()

    with tile.TileContext(nc) as tc, ExitStack() as ctx:
        consts = ctx.enter_context(tc.tile_pool(name="consts", bufs=1))
        sb = ctx.enter_context(tc.tile_pool(name="sb", bufs=1))
        ps = ctx.enter_context(tc.tile_pool(name="ps", bufs=2, space="PSUM"))
        dram = ctx.enter_context(tc.tile_pool(name="dram", bufs=1,
                                              space="DRAM"))

        wt = consts.tile([128, WCOLS], F32R)
        nc.sync.dma_start(wt[:, 0:448], w_d[:, 0:448])
        nc.scalar.dma_start(wt[:, 448:WCOLS], w_d[:, 448:WCOLS])
        ones_f32 = consts.tile([1, 512], F32)
        nc.vector.memset(ones_f32[:], 1.0)
        ones512 = consts.tile([1, 512], F32R)
        nc.vector.tensor_copy(ones512[:], ones_f32[:])
        ones128 = consts.tile([1, 128], F32R)
        nc.vector.tensor_copy(ones128[:], ones_f32[:, 0:128])
        pools = (sb, ps, wt[:], ones512, ones128)

        # spread big loads across the three DMA-dispatch queues
        dma_engines = [nc.sync, nc.scalar, nc.gpsimd]

        for _rep in range(reps):
            xT = sb.tile([64, S], F32R, tag="xt", bufs=2, name=f"xT_{_rep}")
            for n in range(S // 512):
                dma_engines[n % 3].dma_start(
                    xT[:, n * 512:(n + 1) * 512],
                    xT_d[:, n * 512:(n + 1) * 512])

            outT = sb.tile([64, R], F32R, tag="outT")
            xT2 = sb.tile([64, S], F32R, tag="xt", bufs=2, name=f"xT2_{_rep}")
            csz = R // exch_chunks
            bounce_ins = [dram.tile([64, csz], F32R,
                                    name=f"bi_{_rep}_{n}", tag=f"bi{n}")
                          for n in range(exch_chunks)]
            bounce_outs = [dram.tile([2, 64, csz], F32R,
                                     name=f"bo_{_rep}_{n}", tag=f"bo{n}")
                           for n in range(exch_chunks)]

            def exchange_chunk(n):
                # fire the exchange for every bounce chunk fully covered by
                # the just-finished outT chunk n (chunk size 512), then pull
                # both gathered halves straight into xT2
                for e in range(exch_chunks):
                    if (e + 1) * csz <= (n + 1) * 512 and \
                            (e + 1) * csz > n * 512:
                        sl = slice(e * csz, (e + 1) * csz)
                        nc.sync.dma_start(bounce_ins[e][:], outT[:, sl])
                        if n_cores > 1:
                            nc.gpsimd.collective_compute(
                                "AllGather", mybir.AluOpType.bypass,
                                replica_groups=[[0, 1], [2, 3],
                                                [4, 5], [6, 7]],
                                ins=[bounce_ins[e][:].opt()],
                                outs=[bounce_outs[e][:].opt()])
                        else:
                            for m in range(2):
                                nc.sync.dma_start(bounce_outs[e][m],
                                                  bounce_ins[e][:])
                        for m in range(2):
                            dma_engines[(m * exch_chunks + e) % 3].dma_start(
                                xT2[:, m * R + e * csz:m * R + (e + 1) * csz],
                                bounce_outs[e][m])

            proj1 = make_proj(nc, pools, 0, xT[:], xT[:, 0:R])
            proj2 = make_proj(nc, pools, 1, xT2[:], outT[:])
            emit2 = proj2[3]
            _stage(nc, pools, 0, proj1, list(range(8)), outT=outT,
                   out_chunk_hook=exchange_chunk)
            _stage(nc, pools, 1, proj2, list(range(8)), out_dram=out_d)

    nc.compile()
    return nc


def prep_inputs(x, q, k, v, q1, k1, v1, W1, b1, W2, b2, W11, b11, W22, b22):
    """Returns per-core in_maps for run_bass_kernel_spmd."""
    f = np.float32

    def cast(a):
        return np.ascontiguousarray(np.asarray(a), dtype=f)

    scale = f(1.0 / np.sqrt(np.float32(64)))
    wpack = np.zeros((128, WCOLS), dtype=f)
    for sfx, (qq, kk, vv, W1_, b1_, W2_, b2_) in enumerate(
            [(q, k, v, W1, b1, W2, b2), (q1, k1, v1, W11, b11, W22, b22)]):
        c0 = 448 * sfx
        wpack[0:64, c0 + WQ0:c0 + WQ0 + 64] = cast(qq) * scale
        wpack[0:64, c0 + WK0:c0 + WK0 + 64] = cast(kk)
        wpack[0:64, c0 + WV0:c0 + WV0 + 64] = cast(vv)
        wpack[0:64, c0 + W1T0:c0 + W1T0 + HD] = cast(W1_).T
        w2T = cast(W2_).T                                 # [HD, 64]
        for j in range(2):
            wpack[:, W2T0 + sfx * 128 + j * 64:
                  W2T0 + sfx * 128 + (j + 1) * 64] = w2T[j * 128:(j + 1) * 128]
            wpack[:, B1C0 + sfx * 2 + j] = cast(b1_)[j * 128:(j + 1) * 128]
        wpack[0, B2R0 + sfx * 64:B2R0 + (sfx + 1) * 64] = \
            cast(b2_) - cast(W2_).sum(axis=1)

    in_maps = []
    xc = cast(x)
    for c in range(N_CORES):
        b, h = c // 2, c % 2
        xb = xc[b]                      # [S, 64]
        if h == 1:                      # own half first
            xb = np.concatenate([xb[R:], xb[:R]], axis=0)
        in_maps.append({"xT": np.ascontiguousarray(xb.T), "wpack": wpack})
    return in_maps


_NC_CACHE = None


def kernel(**inputs) -> np.ndarray:
    global _NC_CACHE
    if _NC_CACHE is None:
        _NC_CACHE = build_nc()
    nc = _NC_CACHE
    in_maps = prep_inputs(**inputs)
    res = bass_utils.run_bass_kernel_spmd(nc, in_maps,
                                          core_ids=list(range(N_CORES)))
    out = np.empty((B, S, 64), dtype=np.float32)
    for c in range(N_CORES):
        b, h = c // 2, c % 2
        out[b, h * R:(h + 1) * R, :] = res.results[c]["out1"]
    return out
